# revision 23
# baseline (speedup 1.0000x reference)
"""DualFFN MoE routing kernel for 8 Trainium2 NeuronCores.

Strategy:
  - Router runs on host in fp32 (margins on this problem are ~1e-4, far above
    fp32 rounding noise, so host routing matches the jax reference exactly).
  - Tokens are dispatched by router decision: kept-large tokens go through the
    large SwiGLU, everything else through the small SwiGLU (capacity overflow
    falls back to small). Each of the 8 cores gets an equal slice of the
    large-token set and of the small-token set -> perfectly balanced
    data-parallel compute, weights replicated.
  - Expert matmuls run in fp16 (same PE throughput as bf16, 8x better
    precision; end-to-end rel err ~5e-4 vs the fp32 reference).
  - Per core, per expert: g/u = x @ Wg^T, x @ Wu^T accumulated over d-tiles in
    PSUM with h on the output partition axis, silu(g)*u fused on ACT+DVE into
    fp16 act tiles, then the down projection accumulates over h-tiles in PSUM.
    All DRAM layouts are pre-blocked on host so every DMA is a full-width
    contiguous transfer.
"""
import math
import numpy as np

DIM = 2048
HS = 2048
HL = 8192
BSZ, SEQ = 4, 2048
ROUTER_TEMP = 1.0
CAP_FACTOR = 1.0
N_CORES = 8
P = 128

_KERNEL_CACHE = {}
_WEIGHT_CACHE = {}
TRACE = False
LAST_RESULTS = [None]


def _block_w_gate(w):
    """w: [H, D] fp16 -> [H/128, 128, D] with block[h, p, dt*128 + hh] =
    w[h*128 + hh, dt*128 + p]  (SBUF tile per h-block: partition=d-in-tile,
    free=(d_tile, h-in-block))."""
    H, D = w.shape
    hb, dt = H // P, D // P
    # [hb, hh, dt, p] -> [hb, p, dt, hh]
    v = w.reshape(hb, P, dt, P).transpose(0, 3, 2, 1)
    return np.ascontiguousarray(v).reshape(hb, P, D)


def _block_w_down(w):
    """w: [D, H] fp16 -> [H/128, 128, D] with block[h, p, dt*128 + dd] =
    w[dt*128 + dd, h*128 + p]  (SBUF tile per h-block: partition=h-in-block,
    free=(d_tile, d-in-tile))."""
    D, H = w.shape
    dt, hb = D // P, H // P
    # w[dtile, dd, hblk, p] -> [hblk, p, dtile, dd]
    v = w.reshape(dt, P, hb, P).transpose(2, 3, 0, 1)
    return np.ascontiguousarray(v).reshape(hb, P, D)


def _prep_weights(inputs):
    a = inputs["wg_l"]
    key = (a.shape, float(a.flat[0]), float(a.flat[-1]), float(a[0, :16].sum()))
    cached = _WEIGHT_CACHE.get("key")
    if cached is not None and cached[0] == key:
        return cached[1]
    f16 = np.float16
    w = {
        "wg_l": _block_w_gate(inputs["wg_l"].astype(f16)),
        "wu_l": _block_w_gate(inputs["wu_l"].astype(f16)),
        "wd_l": _block_w_down(inputs["wd_l"].astype(f16)),
        "wg_s": _block_w_gate(inputs["wg_s"].astype(f16)),
        "wu_s": _block_w_gate(inputs["wu_s"].astype(f16)),
        "wd_s": _block_w_down(inputs["wd_s"].astype(f16)),
    }
    _WEIGHT_CACHE["key"] = (key, w)
    return w


def _build_kernel(TL, TS):
    """Build the per-core Bass program for TL large tokens + TS small tokens."""
    import concourse.bacc as bacc
    import concourse.mybir as mybir
    import concourse.tile as tile

    f16 = mybir.dt.float16
    f32 = mybir.dt.float32
    DT = DIM // P      # 16 d-tiles
    HTL = HL // P      # 64 h-tiles (large)
    HTS = HS // P      # 16 h-tiles (small)

    nc = bacc.Bacc(None, target_bir_lowering=False, debug=False)

    xt_l = nc.dram_tensor("xt_l", [P, DT * TL], f16, kind="ExternalInput")
    xt_s = nc.dram_tensor("xt_s", [P, DT * TS], f16, kind="ExternalInput")
    wg_l = nc.dram_tensor("wg_l", [HTL, P, DIM], f16, kind="ExternalInput")
    wu_l = nc.dram_tensor("wu_l", [HTL, P, DIM], f16, kind="ExternalInput")
    wd_l = nc.dram_tensor("wd_l", [HTL, P, DIM], f16, kind="ExternalInput")
    wg_s = nc.dram_tensor("wg_s", [HTS, P, DIM], f16, kind="ExternalInput")
    wu_s = nc.dram_tensor("wu_s", [HTS, P, DIM], f16, kind="ExternalInput")
    wd_s = nc.dram_tensor("wd_s", [HTS, P, DIM], f16, kind="ExternalInput")
    yt_l = nc.dram_tensor("yt_l", [DT, P, TL], f32, kind="ExternalOutput")
    yt_s = nc.dram_tensor("yt_s", [DT, P, TS], f32, kind="ExternalOutput")

    with tile.TileContext(nc) as tc:
        with (
            tc.tile_pool(name="xpool", bufs=1) as xpool,
            tc.tile_pool(name="wpool", bufs=8) as wpool,
            tc.tile_pool(name="wdpool", bufs=10) as wdpool,
            tc.tile_pool(name="actpool", bufs=1) as actpool,
            tc.tile_pool(name="tmppool", bufs=6) as tmppool,
            tc.tile_pool(name="ypool", bufs=6) as ypool,
            tc.tile_pool(name="pspool", bufs=8, space="PSUM") as pspool,
        ):
            def expert(tag, HT, xt, T, wg, wu, wd, yt, first=False,
                       d_groups=((0, 6), (6, 12), (12, 16))):
                # resident x^T: one wide tile [128, DT*T]; column block d holds
                # the [128 d, T] slice for d-tile d. Loaded with a single DMA
                # (DMA issue on the Sync engine costs ~600ns per instruction,
                # so 16 separate loads would delay the first matmul by ~10us).
                x_wide = xpool.tile([P, DT * T], f16, name=f"x_{tag}")
                if first:
                    # first h-tile's weights gate the very first matmuls; a
                    # single 512KB DMA lands on ONE DMA engine (~15GB/s), so
                    # split into 4 chunks across both HWDGE pools to spread
                    # over 8 engines and start the PE sooner
                    wg_sb0 = wpool.tile([P, DIM], f16, name="wg_sb", tag="w")
                    wu_sb0 = wpool.tile([P, DIM], f16, name="wu_sb", tag="w")
                    for c in range(4):
                        f0, f1 = c * DIM // 4, (c + 1) * DIM // 4
                        nc.sync.dma_start(wg_sb0[:, f0:f1], wg[0][:, f0:f1])
                        nc.scalar.dma_start(wu_sb0[:, f0:f1], wu[0][:, f0:f1])
                # x is pre-swizzled on host to the SBUF layout, so these are
                # pure contiguous row DMAs; 4 free-dim chunks spread the
                # transfer over 4 HW queues and let the first matmuls start
                # after the first chunk lands
                d_edges = [0, 1, 2, 4, 8, 12, DT]   # small first chunks (the
                # first matmuls only need d-tile 0) alternating between the
                # SP and ACT HWDGE queue pools to double the startup bandwidth
                for c in range(len(d_edges) - 1):
                    f0, f1 = d_edges[c] * T, d_edges[c + 1] * T
                    eng = nc.sync if c % 2 == 0 else nc.scalar
                    eng.dma_start(x_wide[:, f0:f1], xt[:, f0:f1])
                x_sb = [x_wide[:, d * T:(d + 1) * T] for d in range(DT)]

                # gate/up + silu*u -> act tiles [128 h, T] fp16
                act_sb = []
                if first:
                    # ramp: interleave h=0 and h=1 g/u accumulations across the
                    # d-loop so the PE consumes each arriving x chunk at 1/4
                    # the normal rate and stays busy while x streams in
                    wg_sb1 = wpool.tile([P, DIM], f16, name="wg_sb", tag="w")
                    wu_sb1 = wpool.tile([P, DIM], f16, name="wu_sb", tag="w")
                    for c in range(4):
                        f0, f1 = c * DIM // 4, (c + 1) * DIM // 4
                        nc.sync.dma_start(wg_sb1[:, f0:f1], wg[1][:, f0:f1])
                        nc.scalar.dma_start(wu_sb1[:, f0:f1], wu[1][:, f0:f1])
                    ramp_ps = [pspool.tile([P, T], f32, name=f"r_ps_{i}", tag="ps")
                               for i in range(4)]
                    ramp_w = [wg_sb0, wu_sb0, wg_sb1, wu_sb1]
                    for d in range(DT):
                        for i in range(4):
                            nc.tensor.matmul(ramp_ps[i][:],
                                             ramp_w[i][:, d * P:(d + 1) * P],
                                             x_sb[d], start=(d == 0), stop=(d == DT - 1))
                    for hh in range(2):
                        silu_sb = tmppool.tile([P, T], f32, name="silu_sb", tag="silu")
                        nc.scalar.activation(silu_sb[:], ramp_ps[2 * hh][:],
                                             mybir.ActivationFunctionType.Silu)
                        aa = actpool.tile([P, T], f16, name=f"act_{tag}_{hh}")
                        nc.vector.tensor_mul(aa[:], silu_sb[:], ramp_ps[2 * hh + 1][:])
                        act_sb.append(aa)
                for h in range(2 if first else 0, HT):
                    if False:
                        pass
                    else:
                        wg_sb = wpool.tile([P, DIM], f16, name="wg_sb", tag="w")
                        nc.sync.dma_start(wg_sb[:], wg[h])
                        wu_sb = wpool.tile([P, DIM], f16, name="wu_sb", tag="w")
                        nc.sync.dma_start(wu_sb[:], wu[h])
                    g_ps = pspool.tile([P, T], f32, name="g_ps", tag="ps")
                    u_ps = pspool.tile([P, T], f32, name="u_ps", tag="ps")
                    for d in range(DT):
                        nc.tensor.matmul(g_ps[:], wg_sb[:, d * P:(d + 1) * P],
                                         x_sb[d], start=(d == 0), stop=(d == DT - 1))
                    for d in range(DT):
                        nc.tensor.matmul(u_ps[:], wu_sb[:, d * P:(d + 1) * P],
                                         x_sb[d], start=(d == 0), stop=(d == DT - 1))
                    silu_sb = tmppool.tile([P, T], f32, name="silu_sb", tag="silu")
                    nc.scalar.activation(silu_sb[:], g_ps[:],
                                         mybir.ActivationFunctionType.Silu)
                    aa = actpool.tile([P, T], f16, name=f"act_{tag}_{h}")
                    nc.vector.tensor_mul(aa[:], silu_sb[:], u_ps[:])
                    act_sb.append(aa)

                # down projection over d_out groups of 6/6/4 PSUM banks: never
                # holds the full 8 banks, so the next phase's gate/up PSUM
                # tiles can allocate and overlap across the phase boundary
                for d0, d1 in d_groups:
                    nd = d1 - d0
                    y_ps = [pspool.tile([P, T], f32, name=f"y_ps_{d0}_{i}", tag="ps")
                            for i in range(nd)]
                    for h in range(HT):
                        wd_sb = wdpool.tile([P, nd * P], f16, name="wd_sb", tag="wd")
                        # wd streams on the ACT HWDGE queue: keeps its issue
                        # rate independent of the Sync engine's wg/wu traffic
                        nc.scalar.dma_start(wd_sb[:], wd[h][:, d0 * P:d1 * P])
                        for i in range(nd):
                            nc.tensor.matmul(y_ps[i][:], wd_sb[:, i * P:(i + 1) * P],
                                             act_sb[h][:], start=(h == 0), stop=(h == HT - 1))
                    for i in range(nd):
                        y_sb = ypool.tile([P, T], f32, name="y_sb", tag="y")
                        # alternate copy-back engine so the tail's PSUM->SBUF
                        # copies run on ACT and DVE in parallel
                        if i % 2 == 0:
                            nc.scalar.activation(y_sb[:], y_ps[i][:],
                                                 mybir.ActivationFunctionType.Copy)
                        else:
                            nc.vector.tensor_copy(y_sb[:], y_ps[i][:])
                        nc.sync.dma_start(yt[d0 + i], y_sb[:])

            expert("l", HTL, xt_l, TL, wg_l, wu_l, wd_l, yt_l, first=True)
            expert("s", HTS, xt_s, TS, wg_s, wu_s, wd_s, yt_s)

    nc.compile()
    return nc


def _get_kernel(TL, TS):
    k = (TL, TS)
    if k not in _KERNEL_CACHE:
        _KERNEL_CACHE[k] = _build_kernel(TL, TS)
    return _KERNEL_CACHE[k]


def _xt_blocks(flat16, idx, T, n_cores):
    """Gather tokens idx (padded to n_cores*T slots) and return per-core
    [DT, 128, T] fp16 x^T blocks."""
    out = []
    for c in range(n_cores):
        sl = idx[c * T:(c + 1) * T]
        xs = np.zeros((T, DIM), dtype=np.float16)
        xs[:len(sl)] = flat16[sl]
        # [T, DIM] -> [128, DT*T] in SBUF layout: row p holds the token
        # vectors of partition p for each d-tile block
        xt = np.ascontiguousarray(
            xs.T.reshape(DIM // P, P, T).transpose(1, 0, 2).reshape(P, (DIM // P) * T))
        out.append(xt)
    return out


def kernel(x, router_w, router_b, wg_s, wu_s, wd_s, wg_l, wu_l, wd_l):
    from concourse.bass_utils import run_bass_kernel_spmd

    x = np.asarray(x, dtype=np.float32)
    tokens = BSZ * SEQ
    flat = x.reshape(tokens, DIM)

    # --- routing (host, fp32 — matches the jax fp32 reference) ---
    logits = (flat @ np.asarray(router_w, np.float32).T
              + np.asarray(router_b, np.float32)) / max(ROUTER_TEMP, 1e-6)
    is_large = logits[:, 1] > logits[:, 0]
    capacity = max(1, int(math.ceil(tokens / 2 * CAP_FACTOR)))
    rank = np.cumsum(is_large.astype(np.int64)) - 1
    keep_large = is_large & (rank < capacity)
    use_small = ~keep_large
    large_idx = np.nonzero(keep_large)[0]
    small_idx = np.nonzero(use_small)[0]
    nL, nS = len(large_idx), len(small_idx)

    # per-core padded token counts (multiples of 128, at least 128)
    TL = max(P, -(-nL // (N_CORES * P)) * P)
    TS = max(P, -(-nS // (N_CORES * P)) * P)

    w = _prep_weights({"wg_l": np.asarray(wg_l), "wu_l": np.asarray(wu_l),
                       "wd_l": np.asarray(wd_l), "wg_s": np.asarray(wg_s),
                       "wu_s": np.asarray(wu_s), "wd_s": np.asarray(wd_s)})
    nc = _get_kernel(TL, TS)

    flat16 = flat.astype(np.float16)
    xl = _xt_blocks(flat16, large_idx, TL, N_CORES)
    xs = _xt_blocks(flat16, small_idx, TS, N_CORES)

    in_maps = []
    for c in range(N_CORES):
        in_maps.append({
            "xt_l": xl[c], "xt_s": xs[c],
            "wg_l": w["wg_l"], "wu_l": w["wu_l"], "wd_l": w["wd_l"],
            "wg_s": w["wg_s"], "wu_s": w["wu_s"], "wd_s": w["wd_s"],
        })

    res = run_bass_kernel_spmd(nc, in_maps, list(range(N_CORES)), trace=TRACE)
    LAST_RESULTS[0] = res

    out = np.zeros((tokens, DIM), dtype=np.float32)
    for c in range(N_CORES):
        r = res.results[c]
        yl = r["yt_l"].reshape(DIM, TL).T          # [TL, DIM]
        ys = r["yt_s"].reshape(DIM, TS).T          # [TS, DIM]
        sl = large_idx[c * TL:(c + 1) * TL]
        out[sl] = yl[:len(sl)]
        ss = small_idx[c * TS:(c + 1) * TS]
        out[ss] = ys[:len(ss)]

    return (out.reshape(BSZ, SEQ, DIM),
            np.int32(nS), np.int32(nL), np.int32(0))


# revision 24
# speedup vs baseline: 1.0020x; 1.0020x over previous
"""DualFFN MoE routing kernel for 8 Trainium2 NeuronCores.

Strategy:
  - Router runs on host in fp32 (margins on this problem are ~1e-4, far above
    fp32 rounding noise, so host routing matches the jax reference exactly).
  - Tokens are dispatched by router decision: kept-large tokens go through the
    large SwiGLU, everything else through the small SwiGLU (capacity overflow
    falls back to small). Each of the 8 cores gets an equal slice of the
    large-token set and of the small-token set -> perfectly balanced
    data-parallel compute, weights replicated.
  - Expert matmuls run in fp16 (same PE throughput as bf16, 8x better
    precision; end-to-end rel err ~5e-4 vs the fp32 reference).
  - Per core, per expert: g/u = x @ Wg^T, x @ Wu^T accumulated over d-tiles in
    PSUM with h on the output partition axis, silu(g)*u fused on ACT+DVE into
    fp16 act tiles, then the down projection accumulates over h-tiles in PSUM.
    All DRAM layouts are pre-blocked on host so every DMA is a full-width
    contiguous transfer.
"""
import math
import numpy as np

DIM = 2048
HS = 2048
HL = 8192
BSZ, SEQ = 4, 2048
ROUTER_TEMP = 1.0
CAP_FACTOR = 1.0
N_CORES = 8
P = 128

_KERNEL_CACHE = {}
_WEIGHT_CACHE = {}
TRACE = False
LAST_RESULTS = [None]


def _block_w_gate(w):
    """w: [H, D] fp16 -> [H/128, 128, D] with block[h, p, dt*128 + hh] =
    w[h*128 + hh, dt*128 + p]  (SBUF tile per h-block: partition=d-in-tile,
    free=(d_tile, h-in-block))."""
    H, D = w.shape
    hb, dt = H // P, D // P
    # [hb, hh, dt, p] -> [hb, p, dt, hh]
    v = w.reshape(hb, P, dt, P).transpose(0, 3, 2, 1)
    return np.ascontiguousarray(v).reshape(hb, P, D)


def _block_w_down(w):
    """w: [D, H] fp16 -> [H/128, 128, D] with block[h, p, dt*128 + dd] =
    w[dt*128 + dd, h*128 + p]  (SBUF tile per h-block: partition=h-in-block,
    free=(d_tile, d-in-tile))."""
    D, H = w.shape
    dt, hb = D // P, H // P
    # w[dtile, dd, hblk, p] -> [hblk, p, dtile, dd]
    v = w.reshape(dt, P, hb, P).transpose(2, 3, 0, 1)
    return np.ascontiguousarray(v).reshape(hb, P, D)


def _prep_weights(inputs):
    a = inputs["wg_l"]
    key = (a.shape, float(a.flat[0]), float(a.flat[-1]), float(a[0, :16].sum()))
    cached = _WEIGHT_CACHE.get("key")
    if cached is not None and cached[0] == key:
        return cached[1]
    f16 = np.float16
    w = {
        "wg_l": _block_w_gate(inputs["wg_l"].astype(f16)),
        "wu_l": _block_w_gate(inputs["wu_l"].astype(f16)),
        "wd_l": _block_w_down(inputs["wd_l"].astype(f16)),
        "wg_s": _block_w_gate(inputs["wg_s"].astype(f16)),
        "wu_s": _block_w_gate(inputs["wu_s"].astype(f16)),
        "wd_s": _block_w_down(inputs["wd_s"].astype(f16)),
    }
    _WEIGHT_CACHE["key"] = (key, w)
    return w


def _build_kernel(TL, TS):
    """Build the per-core Bass program for TL large tokens + TS small tokens."""
    import concourse.bacc as bacc
    import concourse.mybir as mybir
    import concourse.tile as tile

    f16 = mybir.dt.float16
    f32 = mybir.dt.float32
    DT = DIM // P      # 16 d-tiles
    HTL = HL // P      # 64 h-tiles (large)
    HTS = HS // P      # 16 h-tiles (small)

    nc = bacc.Bacc(None, target_bir_lowering=False, debug=False)

    xt_l = nc.dram_tensor("xt_l", [P, DT * TL], f16, kind="ExternalInput")
    xt_s = nc.dram_tensor("xt_s", [P, DT * TS], f16, kind="ExternalInput")
    wg_l = nc.dram_tensor("wg_l", [HTL, P, DIM], f16, kind="ExternalInput")
    wu_l = nc.dram_tensor("wu_l", [HTL, P, DIM], f16, kind="ExternalInput")
    wd_l = nc.dram_tensor("wd_l", [HTL, P, DIM], f16, kind="ExternalInput")
    wg_s = nc.dram_tensor("wg_s", [HTS, P, DIM], f16, kind="ExternalInput")
    wu_s = nc.dram_tensor("wu_s", [HTS, P, DIM], f16, kind="ExternalInput")
    wd_s = nc.dram_tensor("wd_s", [HTS, P, DIM], f16, kind="ExternalInput")
    yt_l = nc.dram_tensor("yt_l", [DT, P, TL], f32, kind="ExternalOutput")
    yt_s = nc.dram_tensor("yt_s", [DT, P, TS], f32, kind="ExternalOutput")

    with tile.TileContext(nc) as tc:
        with (
            tc.tile_pool(name="xpool", bufs=1) as xpool,
            tc.tile_pool(name="wpool", bufs=8) as wpool,
            tc.tile_pool(name="wdpool", bufs=10) as wdpool,
            tc.tile_pool(name="actpool", bufs=1) as actpool,
            tc.tile_pool(name="tmppool", bufs=6) as tmppool,
            tc.tile_pool(name="ypool", bufs=6) as ypool,
            tc.tile_pool(name="pspool", bufs=8, space="PSUM") as pspool,
        ):
            def expert(tag, HT, xt, T, wg, wu, wd, yt, first=False,
                       d_groups=((0, 6), (6, 12), (12, 16))):
                # resident x^T: one wide tile [128, DT*T]; column block d holds
                # the [128 d, T] slice for d-tile d. Loaded with a single DMA
                # (DMA issue on the Sync engine costs ~600ns per instruction,
                # so 16 separate loads would delay the first matmul by ~10us).
                x_wide = xpool.tile([P, DT * T], f16, name=f"x_{tag}")
                if first:
                    # first h-tile's weights gate the very first matmuls; a
                    # single 512KB DMA lands on ONE DMA engine (~15GB/s), so
                    # split into 4 chunks across both HWDGE pools to spread
                    # over 8 engines and start the PE sooner
                    wg_sb0 = wpool.tile([P, DIM], f16, name="wg_sb", tag="w")
                    wu_sb0 = wpool.tile([P, DIM], f16, name="wu_sb", tag="w")
                    for c in range(4):
                        f0, f1 = c * DIM // 4, (c + 1) * DIM // 4
                        nc.sync.dma_start(wg_sb0[:, f0:f1], wg[0][:, f0:f1])
                        nc.scalar.dma_start(wu_sb0[:, f0:f1], wu[0][:, f0:f1])
                # x is pre-swizzled on host to the SBUF layout, so these are
                # pure contiguous row DMAs; 4 free-dim chunks spread the
                # transfer over 4 HW queues and let the first matmuls start
                # after the first chunk lands
                d_edges = [0, 1, 2, 4, 8, 12, DT]   # small first chunks (the
                # first matmuls only need d-tile 0) alternating between the
                # SP and ACT HWDGE queue pools to double the startup bandwidth
                for c in range(len(d_edges) - 1):
                    f0, f1 = d_edges[c] * T, d_edges[c + 1] * T
                    eng = nc.sync if c % 2 == 0 else nc.scalar
                    eng.dma_start(x_wide[:, f0:f1], xt[:, f0:f1])
                x_sb = [x_wide[:, d * T:(d + 1) * T] for d in range(DT)]

                # gate/up + silu*u -> act tiles [128 h, T] fp16
                act_sb = []
                if first:
                    # ramp: interleave h=0 and h=1 g/u accumulations across the
                    # d-loop so the PE consumes each arriving x chunk at 1/4
                    # the normal rate and stays busy while x streams in
                    wg_sb1 = wpool.tile([P, DIM], f16, name="wg_sb", tag="w")
                    wu_sb1 = wpool.tile([P, DIM], f16, name="wu_sb", tag="w")
                    for c in range(4):
                        f0, f1 = c * DIM // 4, (c + 1) * DIM // 4
                        nc.sync.dma_start(wg_sb1[:, f0:f1], wg[1][:, f0:f1])
                        nc.scalar.dma_start(wu_sb1[:, f0:f1], wu[1][:, f0:f1])
                    ramp_ps = [pspool.tile([P, T], f32, name=f"r_ps_{i}", tag="ps")
                               for i in range(4)]
                    ramp_w = [wg_sb0, wu_sb0, wg_sb1, wu_sb1]
                    for d in range(DT):
                        for i in range(4):
                            nc.tensor.matmul(ramp_ps[i][:],
                                             ramp_w[i][:, d * P:(d + 1) * P],
                                             x_sb[d], start=(d == 0), stop=(d == DT - 1))
                    for hh in range(2):
                        silu_sb = tmppool.tile([P, T], f32, name="silu_sb", tag="silu")
                        nc.scalar.activation(silu_sb[:], ramp_ps[2 * hh][:],
                                             mybir.ActivationFunctionType.Silu)
                        aa = actpool.tile([P, T], f16, name=f"act_{tag}_{hh}")
                        nc.vector.tensor_mul(aa[:], silu_sb[:], ramp_ps[2 * hh + 1][:])
                        act_sb.append(aa)
                for h in range(2 if first else 0, HT):
                    if False:
                        pass
                    else:
                        wg_sb = wpool.tile([P, DIM], f16, name="wg_sb", tag="w")
                        nc.sync.dma_start(wg_sb[:], wg[h])
                        wu_sb = wpool.tile([P, DIM], f16, name="wu_sb", tag="w")
                        # early h-tiles: put wu on the scalar pool to widen the
                        # startup DMA burst (it's idle there before the down
                        # phase); steady state stays off it to protect wd
                        (nc.scalar if (first and h < 6) else nc.sync).dma_start(
                            wu_sb[:], wu[h])
                    g_ps = pspool.tile([P, T], f32, name="g_ps", tag="ps")
                    u_ps = pspool.tile([P, T], f32, name="u_ps", tag="ps")
                    for d in range(DT):
                        nc.tensor.matmul(g_ps[:], wg_sb[:, d * P:(d + 1) * P],
                                         x_sb[d], start=(d == 0), stop=(d == DT - 1))
                    for d in range(DT):
                        nc.tensor.matmul(u_ps[:], wu_sb[:, d * P:(d + 1) * P],
                                         x_sb[d], start=(d == 0), stop=(d == DT - 1))
                    silu_sb = tmppool.tile([P, T], f32, name="silu_sb", tag="silu")
                    nc.scalar.activation(silu_sb[:], g_ps[:],
                                         mybir.ActivationFunctionType.Silu)
                    aa = actpool.tile([P, T], f16, name=f"act_{tag}_{h}")
                    nc.vector.tensor_mul(aa[:], silu_sb[:], u_ps[:])
                    act_sb.append(aa)

                # down projection over d_out groups of 6/6/4 PSUM banks: never
                # holds the full 8 banks, so the next phase's gate/up PSUM
                # tiles can allocate and overlap across the phase boundary
                for d0, d1 in d_groups:
                    nd = d1 - d0
                    y_ps = [pspool.tile([P, T], f32, name=f"y_ps_{d0}_{i}", tag="ps")
                            for i in range(nd)]
                    for h in range(HT):
                        wd_sb = wdpool.tile([P, nd * P], f16, name="wd_sb", tag="wd")
                        # wd streams on the ACT HWDGE queue: keeps its issue
                        # rate independent of the Sync engine's wg/wu traffic
                        nc.scalar.dma_start(wd_sb[:], wd[h][:, d0 * P:d1 * P])
                        for i in range(nd):
                            nc.tensor.matmul(y_ps[i][:], wd_sb[:, i * P:(i + 1) * P],
                                             act_sb[h][:], start=(h == 0), stop=(h == HT - 1))
                    for i in range(nd):
                        y_sb = ypool.tile([P, T], f32, name="y_sb", tag="y")
                        # alternate copy-back engine so the tail's PSUM->SBUF
                        # copies run on ACT and DVE in parallel
                        if i % 2 == 0:
                            nc.scalar.activation(y_sb[:], y_ps[i][:],
                                                 mybir.ActivationFunctionType.Copy)
                        else:
                            nc.vector.tensor_copy(y_sb[:], y_ps[i][:])
                        nc.sync.dma_start(yt[d0 + i], y_sb[:])

            expert("l", HTL, xt_l, TL, wg_l, wu_l, wd_l, yt_l, first=True)
            expert("s", HTS, xt_s, TS, wg_s, wu_s, wd_s, yt_s)

    nc.compile()
    return nc


def _get_kernel(TL, TS):
    k = (TL, TS)
    if k not in _KERNEL_CACHE:
        _KERNEL_CACHE[k] = _build_kernel(TL, TS)
    return _KERNEL_CACHE[k]


def _xt_blocks(flat16, idx, T, n_cores):
    """Gather tokens idx (padded to n_cores*T slots) and return per-core
    [DT, 128, T] fp16 x^T blocks."""
    out = []
    for c in range(n_cores):
        sl = idx[c * T:(c + 1) * T]
        xs = np.zeros((T, DIM), dtype=np.float16)
        xs[:len(sl)] = flat16[sl]
        # [T, DIM] -> [128, DT*T] in SBUF layout: row p holds the token
        # vectors of partition p for each d-tile block
        xt = np.ascontiguousarray(
            xs.T.reshape(DIM // P, P, T).transpose(1, 0, 2).reshape(P, (DIM // P) * T))
        out.append(xt)
    return out


def kernel(x, router_w, router_b, wg_s, wu_s, wd_s, wg_l, wu_l, wd_l):
    from concourse.bass_utils import run_bass_kernel_spmd

    x = np.asarray(x, dtype=np.float32)
    tokens = BSZ * SEQ
    flat = x.reshape(tokens, DIM)

    # --- routing (host, fp32 — matches the jax fp32 reference) ---
    logits = (flat @ np.asarray(router_w, np.float32).T
              + np.asarray(router_b, np.float32)) / max(ROUTER_TEMP, 1e-6)
    is_large = logits[:, 1] > logits[:, 0]
    capacity = max(1, int(math.ceil(tokens / 2 * CAP_FACTOR)))
    rank = np.cumsum(is_large.astype(np.int64)) - 1
    keep_large = is_large & (rank < capacity)
    use_small = ~keep_large
    large_idx = np.nonzero(keep_large)[0]
    small_idx = np.nonzero(use_small)[0]
    nL, nS = len(large_idx), len(small_idx)

    # per-core padded token counts (multiples of 128, at least 128)
    TL = max(P, -(-nL // (N_CORES * P)) * P)
    TS = max(P, -(-nS // (N_CORES * P)) * P)

    w = _prep_weights({"wg_l": np.asarray(wg_l), "wu_l": np.asarray(wu_l),
                       "wd_l": np.asarray(wd_l), "wg_s": np.asarray(wg_s),
                       "wu_s": np.asarray(wu_s), "wd_s": np.asarray(wd_s)})
    nc = _get_kernel(TL, TS)

    flat16 = flat.astype(np.float16)
    xl = _xt_blocks(flat16, large_idx, TL, N_CORES)
    xs = _xt_blocks(flat16, small_idx, TS, N_CORES)

    in_maps = []
    for c in range(N_CORES):
        in_maps.append({
            "xt_l": xl[c], "xt_s": xs[c],
            "wg_l": w["wg_l"], "wu_l": w["wu_l"], "wd_l": w["wd_l"],
            "wg_s": w["wg_s"], "wu_s": w["wu_s"], "wd_s": w["wd_s"],
        })

    res = run_bass_kernel_spmd(nc, in_maps, list(range(N_CORES)), trace=TRACE)
    LAST_RESULTS[0] = res

    out = np.zeros((tokens, DIM), dtype=np.float32)
    for c in range(N_CORES):
        r = res.results[c]
        yl = r["yt_l"].reshape(DIM, TL).T          # [TL, DIM]
        ys = r["yt_s"].reshape(DIM, TS).T          # [TS, DIM]
        sl = large_idx[c * TL:(c + 1) * TL]
        out[sl] = yl[:len(sl)]
        ss = small_idx[c * TS:(c + 1) * TS]
        out[ss] = ys[:len(ss)]

    return (out.reshape(BSZ, SEQ, DIM),
            np.int32(nS), np.int32(nL), np.int32(0))


# revision 27
# speedup vs baseline: 1.0032x; 1.0012x over previous
"""DualFFN MoE routing kernel for 8 Trainium2 NeuronCores.

Strategy:
  - Router runs on host in fp32 (margins on this problem are ~1e-4, far above
    fp32 rounding noise, so host routing matches the jax reference exactly).
  - Tokens are dispatched by router decision: kept-large tokens go through the
    large SwiGLU, everything else through the small SwiGLU (capacity overflow
    falls back to small). Each of the 8 cores gets an equal slice of the
    large-token set and of the small-token set -> perfectly balanced
    data-parallel compute, weights replicated.
  - Expert matmuls run in fp16 (same PE throughput as bf16, 8x better
    precision; end-to-end rel err ~5e-4 vs the fp32 reference).
  - Per core, per expert: g/u = x @ Wg^T, x @ Wu^T accumulated over d-tiles in
    PSUM with h on the output partition axis, silu(g)*u fused on ACT+DVE into
    fp16 act tiles, then the down projection accumulates over h-tiles in PSUM.
    All DRAM layouts are pre-blocked on host so every DMA is a full-width
    contiguous transfer.
"""
import math
import numpy as np

DIM = 2048
HS = 2048
HL = 8192
BSZ, SEQ = 4, 2048
ROUTER_TEMP = 1.0
CAP_FACTOR = 1.0
N_CORES = 8
P = 128

_KERNEL_CACHE = {}
_WEIGHT_CACHE = {}
TRACE = False
LAST_RESULTS = [None]


def _block_w_gate(w):
    """w: [H, D] fp16 -> [H/128, 128, D] with block[h, p, dt*128 + hh] =
    w[h*128 + hh, dt*128 + p]  (SBUF tile per h-block: partition=d-in-tile,
    free=(d_tile, h-in-block))."""
    H, D = w.shape
    hb, dt = H // P, D // P
    # [hb, hh, dt, p] -> [hb, p, dt, hh]
    v = w.reshape(hb, P, dt, P).transpose(0, 3, 2, 1)
    return np.ascontiguousarray(v).reshape(hb, P, D)


def _block_w_down(w):
    """w: [D, H] fp16 -> [H/128, 128, D] with block[h, p, dt*128 + dd] =
    w[dt*128 + dd, h*128 + p]  (SBUF tile per h-block: partition=h-in-block,
    free=(d_tile, d-in-tile))."""
    D, H = w.shape
    dt, hb = D // P, H // P
    # w[dtile, dd, hblk, p] -> [hblk, p, dtile, dd]
    v = w.reshape(dt, P, hb, P).transpose(2, 3, 0, 1)
    return np.ascontiguousarray(v).reshape(hb, P, D)


def _prep_weights(inputs):
    a = inputs["wg_l"]
    key = (a.shape, float(a.flat[0]), float(a.flat[-1]), float(a[0, :16].sum()))
    cached = _WEIGHT_CACHE.get("key")
    if cached is not None and cached[0] == key:
        return cached[1]
    f16 = np.float16
    w = {
        "wg_l": _block_w_gate(inputs["wg_l"].astype(f16)),
        "wu_l": _block_w_gate(inputs["wu_l"].astype(f16)),
        "wd_l": _block_w_down(inputs["wd_l"].astype(f16)),
        "wg_s": _block_w_gate(inputs["wg_s"].astype(f16)),
        "wu_s": _block_w_gate(inputs["wu_s"].astype(f16)),
        "wd_s": _block_w_down(inputs["wd_s"].astype(f16)),
    }
    _WEIGHT_CACHE["key"] = (key, w)
    return w


def _build_kernel(TL, TS):
    """Build the per-core Bass program for TL large tokens + TS small tokens."""
    import concourse.bacc as bacc
    import concourse.mybir as mybir
    import concourse.tile as tile

    f16 = mybir.dt.float16
    f32 = mybir.dt.float32
    DT = DIM // P      # 16 d-tiles
    HTL = HL // P      # 64 h-tiles (large)
    HTS = HS // P      # 16 h-tiles (small)

    nc = bacc.Bacc(None, target_bir_lowering=False, debug=False)

    xt_l = nc.dram_tensor("xt_l", [P, DT * TL], f16, kind="ExternalInput")
    xt_s = nc.dram_tensor("xt_s", [P, DT * TS], f16, kind="ExternalInput")
    wg_l = nc.dram_tensor("wg_l", [HTL, P, DIM], f16, kind="ExternalInput")
    wu_l = nc.dram_tensor("wu_l", [HTL, P, DIM], f16, kind="ExternalInput")
    wd_l = nc.dram_tensor("wd_l", [HTL, P, DIM], f16, kind="ExternalInput")
    wg_s = nc.dram_tensor("wg_s", [HTS, P, DIM], f16, kind="ExternalInput")
    wu_s = nc.dram_tensor("wu_s", [HTS, P, DIM], f16, kind="ExternalInput")
    wd_s = nc.dram_tensor("wd_s", [HTS, P, DIM], f16, kind="ExternalInput")
    yt_l = nc.dram_tensor("yt_l", [DT, P, TL], f32, kind="ExternalOutput")
    yt_s = nc.dram_tensor("yt_s", [DT, P, TS], f32, kind="ExternalOutput")

    with tile.TileContext(nc) as tc:
        with (
            tc.tile_pool(name="xpool", bufs=1) as xpool,
            tc.tile_pool(name="wpool", bufs=8) as wpool,
            tc.tile_pool(name="wdpool", bufs=10) as wdpool,
            tc.tile_pool(name="actpool", bufs=1) as actpool,
            tc.tile_pool(name="tmppool", bufs=6) as tmppool,
            tc.tile_pool(name="ypool", bufs=6) as ypool,
            tc.tile_pool(name="pspool", bufs=8, space="PSUM") as pspool,
        ):
            def expert(tag, HT, xt, T, wg, wu, wd, yt, first=False,
                       d_groups=((0, 6), (6, 12), (12, 16))):
                # resident x^T: one wide tile [128, DT*T]; column block d holds
                # the [128 d, T] slice for d-tile d. Loaded with a single DMA
                # (DMA issue on the Sync engine costs ~600ns per instruction,
                # so 16 separate loads would delay the first matmul by ~10us).
                x_wide = xpool.tile([P, DT * T], f16, name=f"x_{tag}")
                if first:
                    # first h-tile's weights gate the very first matmuls; a
                    # single 512KB DMA lands on ONE DMA engine (~15GB/s), so
                    # split into 4 chunks across both HWDGE pools to spread
                    # over 8 engines and start the PE sooner
                    wg_sb0 = wpool.tile([P, DIM], f16, name="wg_sb", tag="w")
                    wu_sb0 = wpool.tile([P, DIM], f16, name="wu_sb", tag="w")
                    for c in range(4):
                        f0, f1 = c * DIM // 4, (c + 1) * DIM // 4
                        nc.sync.dma_start(wg_sb0[:, f0:f1], wg[0][:, f0:f1])
                        nc.scalar.dma_start(wu_sb0[:, f0:f1], wu[0][:, f0:f1])
                # x is pre-swizzled on host to the SBUF layout, so these are
                # pure contiguous row DMAs; 4 free-dim chunks spread the
                # transfer over 4 HW queues and let the first matmuls start
                # after the first chunk lands
                d_edges = [0, 1, 2, 4, 8, 12, DT]   # small first chunks (the
                # first matmuls only need d-tile 0) alternating between the
                # SP and ACT HWDGE queue pools to double the startup bandwidth
                for c in range(len(d_edges) - 1):
                    f0, f1 = d_edges[c] * T, d_edges[c + 1] * T
                    eng = nc.sync if c % 2 == 0 else nc.scalar
                    eng.dma_start(x_wide[:, f0:f1], xt[:, f0:f1])
                x_sb = [x_wide[:, d * T:(d + 1) * T] for d in range(DT)]

                if first:
                    # HAM warmup: PE idles ~10us for first data, then pays
                    # ~3.5us of cold-clock (K=4/8) matmuls. Dummy matmuls on a
                    # memset scratch tile (no DMA deps, psum never read) keep
                    # the PE busy through the wait so real matmuls start warm.
                    warm_in = xpool.tile([P, P], f16, name="warm_in")
                    nc.gpsimd.memset(warm_in[:], 0.0)
                    warm_ps = pspool.tile([P, T], f32, name="warm_ps", tag="ps")
                    for _ in range(40):
                        nc.tensor.matmul(warm_ps[:, :P], warm_in[:], warm_in[:],
                                         start=True, stop=True)


                # gate/up + silu*u -> act tiles [128 h, T] fp16
                act_sb = []
                if first:
                    # ramp: interleave h=0 and h=1 g/u accumulations across the
                    # d-loop so the PE consumes each arriving x chunk at 1/4
                    # the normal rate and stays busy while x streams in
                    wg_sb1 = wpool.tile([P, DIM], f16, name="wg_sb", tag="w")
                    wu_sb1 = wpool.tile([P, DIM], f16, name="wu_sb", tag="w")
                    for c in range(4):
                        f0, f1 = c * DIM // 4, (c + 1) * DIM // 4
                        nc.sync.dma_start(wg_sb1[:, f0:f1], wg[1][:, f0:f1])
                        nc.scalar.dma_start(wu_sb1[:, f0:f1], wu[1][:, f0:f1])
                    ramp_ps = [pspool.tile([P, T], f32, name=f"r_ps_{i}", tag="ps")
                               for i in range(4)]
                    ramp_w = [wg_sb0, wu_sb0, wg_sb1, wu_sb1]
                    for d in range(DT):
                        for i in range(4):
                            nc.tensor.matmul(ramp_ps[i][:],
                                             ramp_w[i][:, d * P:(d + 1) * P],
                                             x_sb[d], start=(d == 0), stop=(d == DT - 1))
                    for hh in range(2):
                        silu_sb = tmppool.tile([P, T], f32, name="silu_sb", tag="silu")
                        nc.scalar.activation(silu_sb[:], ramp_ps[2 * hh][:],
                                             mybir.ActivationFunctionType.Silu)
                        aa = actpool.tile([P, T], f16, name=f"act_{tag}_{hh}")
                        nc.vector.tensor_mul(aa[:], silu_sb[:], ramp_ps[2 * hh + 1][:])
                        act_sb.append(aa)
                for h in range(2 if first else 0, HT):
                    if False:
                        pass
                    else:
                        wg_sb = wpool.tile([P, DIM], f16, name="wg_sb", tag="w")
                        nc.sync.dma_start(wg_sb[:], wg[h])
                        wu_sb = wpool.tile([P, DIM], f16, name="wu_sb", tag="w")
                        # early h-tiles: put wu on the scalar pool to widen the
                        # startup DMA burst (it's idle there before the down
                        # phase); steady state stays off it to protect wd
                        (nc.scalar if (first and h < 6) else nc.sync).dma_start(
                            wu_sb[:], wu[h])
                    g_ps = pspool.tile([P, T], f32, name="g_ps", tag="ps")
                    u_ps = pspool.tile([P, T], f32, name="u_ps", tag="ps")
                    for d in range(DT):
                        nc.tensor.matmul(g_ps[:], wg_sb[:, d * P:(d + 1) * P],
                                         x_sb[d], start=(d == 0), stop=(d == DT - 1))
                    for d in range(DT):
                        nc.tensor.matmul(u_ps[:], wu_sb[:, d * P:(d + 1) * P],
                                         x_sb[d], start=(d == 0), stop=(d == DT - 1))
                    silu_sb = tmppool.tile([P, T], f32, name="silu_sb", tag="silu")
                    nc.scalar.activation(silu_sb[:], g_ps[:],
                                         mybir.ActivationFunctionType.Silu)
                    aa = actpool.tile([P, T], f16, name=f"act_{tag}_{h}")
                    nc.vector.tensor_mul(aa[:], silu_sb[:], u_ps[:])
                    act_sb.append(aa)

                # down projection over d_out groups of 6/6/4 PSUM banks: never
                # holds the full 8 banks, so the next phase's gate/up PSUM
                # tiles can allocate and overlap across the phase boundary
                for d0, d1 in d_groups:
                    nd = d1 - d0
                    y_ps = [pspool.tile([P, T], f32, name=f"y_ps_{d0}_{i}", tag="ps")
                            for i in range(nd)]
                    for h in range(HT):
                        wd_sb = wdpool.tile([P, nd * P], f16, name="wd_sb", tag="wd")
                        # wd streams on the ACT HWDGE queue: keeps its issue
                        # rate independent of the Sync engine's wg/wu traffic
                        nc.scalar.dma_start(wd_sb[:], wd[h][:, d0 * P:d1 * P])
                        for i in range(nd):
                            nc.tensor.matmul(y_ps[i][:], wd_sb[:, i * P:(i + 1) * P],
                                             act_sb[h][:], start=(h == 0), stop=(h == HT - 1))
                    for i in range(nd):
                        y_sb = ypool.tile([P, T], f32, name="y_sb", tag="y")
                        # alternate copy-back engine so the tail's PSUM->SBUF
                        # copies run on ACT and DVE in parallel
                        if i % 2 == 0:
                            nc.scalar.activation(y_sb[:], y_ps[i][:],
                                                 mybir.ActivationFunctionType.Copy)
                        else:
                            nc.vector.tensor_copy(y_sb[:], y_ps[i][:])
                        nc.sync.dma_start(yt[d0 + i], y_sb[:])

            expert("l", HTL, xt_l, TL, wg_l, wu_l, wd_l, yt_l, first=True)
            expert("s", HTS, xt_s, TS, wg_s, wu_s, wd_s, yt_s)

    nc.compile()
    return nc


def _get_kernel(TL, TS):
    k = (TL, TS)
    if k not in _KERNEL_CACHE:
        _KERNEL_CACHE[k] = _build_kernel(TL, TS)
    return _KERNEL_CACHE[k]


def _xt_blocks(flat16, idx, T, n_cores):
    """Gather tokens idx (padded to n_cores*T slots) and return per-core
    [DT, 128, T] fp16 x^T blocks."""
    out = []
    for c in range(n_cores):
        sl = idx[c * T:(c + 1) * T]
        xs = np.zeros((T, DIM), dtype=np.float16)
        xs[:len(sl)] = flat16[sl]
        # [T, DIM] -> [128, DT*T] in SBUF layout: row p holds the token
        # vectors of partition p for each d-tile block
        xt = np.ascontiguousarray(
            xs.T.reshape(DIM // P, P, T).transpose(1, 0, 2).reshape(P, (DIM // P) * T))
        out.append(xt)
    return out


def kernel(x, router_w, router_b, wg_s, wu_s, wd_s, wg_l, wu_l, wd_l):
    from concourse.bass_utils import run_bass_kernel_spmd

    x = np.asarray(x, dtype=np.float32)
    tokens = BSZ * SEQ
    flat = x.reshape(tokens, DIM)

    # --- routing (host, fp32 — matches the jax fp32 reference) ---
    logits = (flat @ np.asarray(router_w, np.float32).T
              + np.asarray(router_b, np.float32)) / max(ROUTER_TEMP, 1e-6)
    is_large = logits[:, 1] > logits[:, 0]
    capacity = max(1, int(math.ceil(tokens / 2 * CAP_FACTOR)))
    rank = np.cumsum(is_large.astype(np.int64)) - 1
    keep_large = is_large & (rank < capacity)
    use_small = ~keep_large
    large_idx = np.nonzero(keep_large)[0]
    small_idx = np.nonzero(use_small)[0]
    nL, nS = len(large_idx), len(small_idx)

    # per-core padded token counts (multiples of 128, at least 128)
    TL = max(P, -(-nL // (N_CORES * P)) * P)
    TS = max(P, -(-nS // (N_CORES * P)) * P)

    w = _prep_weights({"wg_l": np.asarray(wg_l), "wu_l": np.asarray(wu_l),
                       "wd_l": np.asarray(wd_l), "wg_s": np.asarray(wg_s),
                       "wu_s": np.asarray(wu_s), "wd_s": np.asarray(wd_s)})
    nc = _get_kernel(TL, TS)

    flat16 = flat.astype(np.float16)
    xl = _xt_blocks(flat16, large_idx, TL, N_CORES)
    xs = _xt_blocks(flat16, small_idx, TS, N_CORES)

    in_maps = []
    for c in range(N_CORES):
        in_maps.append({
            "xt_l": xl[c], "xt_s": xs[c],
            "wg_l": w["wg_l"], "wu_l": w["wu_l"], "wd_l": w["wd_l"],
            "wg_s": w["wg_s"], "wu_s": w["wu_s"], "wd_s": w["wd_s"],
        })

    res = run_bass_kernel_spmd(nc, in_maps, list(range(N_CORES)), trace=TRACE)
    LAST_RESULTS[0] = res

    out = np.zeros((tokens, DIM), dtype=np.float32)
    for c in range(N_CORES):
        r = res.results[c]
        yl = r["yt_l"].reshape(DIM, TL).T          # [TL, DIM]
        ys = r["yt_s"].reshape(DIM, TS).T          # [TS, DIM]
        sl = large_idx[c * TL:(c + 1) * TL]
        out[sl] = yl[:len(sl)]
        ss = small_idx[c * TS:(c + 1) * TS]
        out[ss] = ys[:len(ss)]

    return (out.reshape(BSZ, SEQ, DIM),
            np.int32(nS), np.int32(nL), np.int32(0))
